# revision 17
# baseline (speedup 1.0000x reference)
"""Trainium2 Bass kernel for nn_AdjLieODEflow (batched 8x8 Lie-ODE RK4 flow).

Math (per sample, 8x8 matrices):
  dU/dt = F(t,U) U,  F = antisym(W0 + t W1 + U W2 U^T) = G(t) + U Wa U^T
  where G(t) = W0a + t W1a (antisym parts), Wa = antisym(W2).
  vel = G U + U (Wa S),  S = U^T U.
  logj rate (Hutchinson, v = eps[0], S=1):
    rate = tr(T Wa S) + tr(Q Wa Q),  T = v^T v,  Q = v^T U
    (the other two JVP-trace terms vanish exactly: <sym, antisym> = 0)
    tr(T Wa S) = <D, S>, D = T Wa (time-constant!) -> accumulated as
      <D, sum_t c_t S_t> with one dot at tile end (Sacc trick).
    tr(Q Wa Q) = <Q2, Q2^T (c Wa)>, Q2 = U^T v, RK weight c folded into Wa.
  RK4, 16 steps, dt = 1/16. Sharding: pure data parallel over 8 cores.

Layout: sample s = tile*2048 + q*128 + p (p = partition, q chunk 0..15).
Each sample's 8x8 matrices live in one partition's free dim (64 elems,
row-major). A per-sample matmul = 8 outer-product elementwise ops (one per
contraction index) into P[q, slot, 8, 8] + a 3-level binary tree of adds.

Everything runs on VectorE. Measured facts that force this design:
 - TensorE can't help: its contraction dim is the partition axis; placing
   per-sample data block-diagonally needs partition-dependent free offsets,
   which no AP can express (BIR: "illegal partition step"), and weight-load
   bandwidth would bottleneck anyway.
 - GpSimd tensor ops ~halve VectorE throughput while active (measured
   2.4x slowdown on independent data), so concurrent GPS offload reduces
   total throughput; GPS is left idle.
 - f32 tensor_tensor runs at 1 elem/cycle (no 2x mode); ops hit the
   formula cost (N+151)/0.96GHz only when the DVE runs alone.
 - bf16 gives 2x only on the unit-stride tree adds; using it for the
   S/H/UH chains blows the 2e-2 error budget (2.8e-2 measured full-size),
   so bf16 (buffers Pb/Tb) is used only for the logj chain (Q2/WQn) and
   the additive velA = G*Ue term (U err 2.5e-4, logj err 4.6e-3).
"""
import sys
import types

sys.path.insert(0, "/opt/trn_rl_repo")
sys.path.insert(0, "/root/.axon_site")

import numpy as np


def _install_profile_hook():
    try:
        import antenv
        if 'antenv.axon_hooks' in sys.modules:
            return
        mod = types.ModuleType('antenv.axon_hooks')
        _h = [None]
        mod.set_axon_ntff_profile_hook = lambda h: _h.__setitem__(0, h)
        mod.get_axon_ntff_profile_hook = lambda: _h[0]
        sys.modules['antenv.axon_hooks'] = mod
        antenv.axon_hooks = mod
        from trn_agent_boot.trn_boot import _ntff_profile_via_ctypes
        hook = _ntff_profile_via_ctypes('/opt/axon/libaxon_pjrt.so')
        if hook is not None:
            mod.set_axon_ntff_profile_hook(hook)
    except Exception:
        pass


_install_profile_hook()

import concourse.bass as bass
import concourse.bacc as bacc
import concourse.tile as tile
from concourse import mybir
from concourse.bass_utils import run_bass_kernel_spmd

f32 = mybir.dt.float32
bf16 = mybir.dt.bfloat16
ADD = mybir.AluOpType.add
MULT = mybir.AluOpType.mult

B = 131072
N_CORES = 8
SHARD = B // N_CORES
NSP = 16
TS = 128 * NSP
N_STEPS = 16
DT = 1.0 / N_STEPS


def _ap(t, dims, off=0):
    base = t[:, :]
    return bass.AP(base.tensor, base.offset + off,
                   [[base.ap[0][0], 128]] + [list(d) for d in dims])


def build(n_cores=N_CORES, shard=SHARD, nsp=NSP, unroll=False,
          state_gps_slots=()):
    ts = 128 * nsp
    tiles = shard // ts
    assert tiles * ts == shard

    nc = bacc.Bacc("TRN2", target_bir_lowering=False, debug=False,
                   num_devices=n_cores)
    u0_d = nc.declare_dram_parameter("u0", [shard, 64], f32, isOutput=False)
    ep_d = nc.declare_dram_parameter("ep", [shard, 64], f32, isOutput=False)
    cst_d = nc.declare_dram_parameter("cst", [39 * 64], f32, isOutput=False)
    uo_d = nc.declare_dram_parameter("uo", [shard, 64], f32, isOutput=True)
    lj_d = nc.declare_dram_parameter("lj", [shard], f32, isOutput=True)

    with tile.TileContext(nc) as tc:
        with tc.tile_pool(name="w", bufs=1) as pool:
            CST = pool.tile([128, 39 * 64], f32)
            U = pool.tile([128, nsp * 64], f32)
            Vv = pool.tile([128, nsp * 64], f32)
            Ue0 = pool.tile([128, nsp * 64], f32)
            Ue1 = pool.tile([128, nsp * 64], f32)
            D = pool.tile([128, nsp * 64], f32)
            Sacc = pool.tile([128, nsp * 64], f32)
            S0 = pool.tile([128, nsp * 64], f32)
            S1 = pool.tile([128, nsp * 64], f32)
            Q2 = pool.tile([128, nsp * 64], bf16)
            H = pool.tile([128, nsp * 64], f32)
            WQn = pool.tile([128, nsp * 64], bf16)
            velA = pool.tile([128, nsp * 64], bf16)
            V1 = pool.tile([128, nsp * 64], f32)
            V2 = pool.tile([128, nsp * 64], f32)
            V3 = pool.tile([128, nsp * 64], f32)
            V4 = pool.tile([128, nsp * 64], f32)
            Aa = pool.tile([128, nsp * 64], f32)
            Ab = pool.tile([128, nsp * 64], f32)
            Pa = pool.tile([128, nsp * 512], f32)
            Ta = pool.tile([128, nsp * 256], f32)
            Pb = pool.tile([128, nsp * 512], bf16)
            Tb = pool.tile([128, nsp * 256], bf16)
            TG = pool.tile([128, nsp * 256], bf16)
            LJ = pool.tile([128, nsp], f32)
            LJs = pool.tile([128, nsp], f32)

            nc.sync.dma_start(out=CST[:, :],
                              in_=bass.AP(cst_d, 0, [[0, 128], [1, 39 * 64]]))

            WA_OFF = 0
            G_OFF = 128
            WAS_OFF = 35 * 64

            def prod_aps(kind, Pbuf, k, inA, inB, offA):
                out = _ap(Pbuf, [(512, nsp), (8, 8), (1, 8)], off=64 * k)
                if kind == 'rowrow':
                    a = _ap(inA, [(64, nsp), (1, 8), (0, 8)], off=8 * k)
                    b = _ap(inB, [(64, nsp), (0, 8), (1, 8)], off=8 * k)
                elif kind == 'constL':
                    a = _ap(inA, [(0, nsp), (8, 8), (0, 8)], off=offA + k)
                    b = _ap(inB, [(64, nsp), (0, 8), (1, 8)], off=8 * k)
                elif kind == 'dataL':
                    a = _ap(inA, [(64, nsp), (8, 8), (0, 8)], off=k)
                    b = _ap(inB, [(64, nsp), (0, 8), (1, 8)], off=8 * k)
                elif kind == 'colconstR':
                    a = _ap(inA, [(64, nsp), (1, 8), (0, 8)], off=8 * k)
                    b = _ap(inB, [(0, nsp), (0, 8), (1, 8)], off=offA + 8 * k)
                elif kind == 'dataLconstR':
                    a = _ap(inA, [(64, nsp), (8, 8), (0, 8)], off=k)
                    b = _ap(inB, [(0, nsp), (0, 8), (1, 8)], off=offA + 8 * k)
                return out, a, b

            def tree(e, Pbuf, Tbuf, out, plus=None):
                e.tensor_tensor(
                    out=_ap(Tbuf, [(256, nsp), (1, 256)]),
                    in0=_ap(Pbuf, [(512, nsp), (1, 256)], off=0),
                    in1=_ap(Pbuf, [(512, nsp), (1, 256)], off=256), op=ADD)
                e.tensor_tensor(
                    out=_ap(Pbuf, [(512, nsp), (1, 128)], off=0),
                    in0=_ap(Tbuf, [(256, nsp), (1, 128)], off=0),
                    in1=_ap(Tbuf, [(256, nsp), (1, 128)], off=128), op=ADD)
                if plus is None:
                    e.tensor_tensor(
                        out=_ap(out, [(64, nsp), (1, 64)]),
                        in0=_ap(Pbuf, [(512, nsp), (1, 64)], off=0),
                        in1=_ap(Pbuf, [(512, nsp), (1, 64)], off=64), op=ADD)
                else:
                    e.tensor_tensor(
                        out=_ap(Tbuf, [(256, nsp), (1, 64)]),
                        in0=_ap(Pbuf, [(512, nsp), (1, 64)], off=0),
                        in1=_ap(Pbuf, [(512, nsp), (1, 64)], off=64), op=ADD)
                    e.tensor_tensor(
                        out=_ap(out, [(64, nsp), (1, 64)]),
                        in0=_ap(Tbuf, [(256, nsp), (1, 64)]),
                        in1=_ap(plus, [(1, nsp * 64)]), op=ADD)

            def dve_group(kind, inA, inB, out, offA=0, plus=None,
                          gps_slots=(), buf=None):
                Pbuf, Tbuf = (Pa, Ta) if buf is None else buf
                for k in range(8):
                    o, a, b = prod_aps(kind, Pbuf, k, inA, inB, offA)
                    e = nc.gpsimd if k in gps_slots else nc.vector
                    e.tensor_tensor(out=o, in0=a, in1=b, op=MULT)
                tree(nc.vector, Pbuf, Tbuf, out, plus=plus)

            def gps_group(kind, inA, inB, out, offA=0):
                for k in range(8):
                    o, a, b = prod_aps(kind, Pb, k, inA, inB, offA)
                    nc.vector.tensor_tensor(out=o, in0=a, in1=b, op=MULT)
                tree(nc.vector, Pb, Tb, out)

            def emit_lj_step():
                nc.vector.tensor_reduce(
                    out=_ap(LJs, [(1, nsp)]),
                    in_=_ap(TG, [(256, nsp), (64, 4), (1, 64)]),
                    axis=mybir.AxisListType.XY, op=ADD)
                nc.vector.tensor_tensor(
                    out=_ap(LJ, [(1, nsp)]),
                    in0=_ap(LJ, [(1, nsp)]),
                    in1=_ap(LJs, [(1, nsp)]), op=ADD)

            def body(ti):
                off_u = ti * (ts * 64)
                off_l = ti * ts
                nc.sync.dma_start(
                    out=_ap(U, [(64, nsp), (1, 64)]),
                    in_=bass.AP(u0_d, off_u, [[64, 128], [8192, nsp], [1, 64]]))
                nc.sync.dma_start(
                    out=_ap(Vv, [(64, nsp), (1, 64)]),
                    in_=bass.AP(ep_d, off_u, [[64, 128], [8192, nsp], [1, 64]]))
                nc.vector.memset(_ap(LJ, [(1, nsp)]), 0.0)
                nc.vector.memset(_ap(Sacc, [(1, nsp * 64)]), 0.0)

                # setup: T = v^T v (H temp), D = T Wa   (GPS, it is idle)
                gps_group('rowrow', Vv, Vv, H)
                gps_group('dataLconstR', H, CST, D, offA=WA_OFF)

                pending = None   # (Ue_tile_or_U, S_tile, st) for logj chain

                def logj_chain(ue_t, s_t, st):
                    gps_group('rowrow', ue_t, Vv, Q2)
                    gps_group('colconstR', Q2, CST, WQn,
                              offA=WAS_OFF + 64 * st)
                    nc.vector.tensor_tensor(
                        out=_ap(TG, [(256, nsp), (1, 64)], off=64 * st),
                        in0=_ap(Q2, [(1, nsp * 64)]),
                        in1=_ap(WQn, [(1, nsp * 64)]), op=MULT)

                for step in range(N_STEPS):
                    for st in range(4):
                        par = (4 * step + st) % 2
                        Scur = (S0, S1)[par]
                        gi = 2 * step + (0 if st == 0 else (1 if st < 3 else 2))
                        if st == 0:
                            Ucur = U
                        else:
                            c = 0.5 * DT if st < 3 else DT
                            Vprev = (V1, V2, V3)[st - 1]
                            Ucur = (Ue0, Ue1)[par]
                            nc.vector.scalar_tensor_tensor(
                                out=_ap(Ucur, [(1, nsp * 64)]),
                                in0=_ap(Vprev, [(1, nsp * 64)]),
                                scalar=float(c),
                                in1=_ap(U, [(1, nsp * 64)]),
                                op0=MULT, op1=ADD)
                        Vst = (V1, V2, V3, V4)[st]

                        dve_group('rowrow', Ucur, Ucur, Scur,
                                  gps_slots=state_gps_slots)
                        dve_group('constL', CST, Scur, H, offA=WA_OFF)
                        dve_group('constL', CST, Ucur, velA,
                                  offA=G_OFF + 64 * gi,
                                  gps_slots=state_gps_slots, buf=(Pb, Tb))
                        dve_group('dataL', Ucur, H, Vst, plus=velA)

                        # term1 accumulation (D const):  Sacc += c_st * S
                        cw = (1.0, 2.0, 2.0, 1.0)[st] * DT / 6.0
                        nc.vector.scalar_tensor_tensor(
                            out=_ap(Sacc, [(1, nsp * 64)]),
                            in0=_ap(Scur, [(1, nsp * 64)]),
                            scalar=float(cw),
                            in1=_ap(Sacc, [(1, nsp * 64)]),
                            op0=MULT, op1=ADD)

                        if st == 1:
                            nc.vector.scalar_tensor_tensor(
                                out=_ap(Aa, [(1, nsp * 64)]),
                                in0=_ap(V2, [(1, nsp * 64)]), scalar=2.0,
                                in1=_ap(V1, [(1, nsp * 64)]),
                                op0=MULT, op1=ADD)
                        elif st == 2:
                            nc.vector.scalar_tensor_tensor(
                                out=_ap(Ab, [(1, nsp * 64)]),
                                in0=_ap(V3, [(1, nsp * 64)]), scalar=2.0,
                                in1=_ap(Aa, [(1, nsp * 64)]),
                                op0=MULT, op1=ADD)
                        elif st == 3:
                            nc.vector.tensor_tensor(
                                out=_ap(Aa, [(1, nsp * 64)]),
                                in0=_ap(V4, [(1, nsp * 64)]),
                                in1=_ap(Ab, [(1, nsp * 64)]), op=ADD)
                            nc.vector.scalar_tensor_tensor(
                                out=_ap(U, [(1, nsp * 64)]),
                                in0=_ap(Aa, [(1, nsp * 64)]),
                                scalar=float(DT / 6.0),
                                in1=_ap(U, [(1, nsp * 64)]),
                                op0=MULT, op1=ADD)

                        # previous step's logj reduce, before slot-0 rewrite
                        if st == 1 and step > 0:
                            emit_lj_step()
                        # lagged logj chain for the previous stage
                        if pending is not None:
                            logj_chain(*pending)
                        pending = (Ucur, Scur, st)

                logj_chain(*pending)
                emit_lj_step()
                # term1 total: LJ += sum(D o Sacc)
                nc.vector.tensor_tensor(
                    out=_ap(Pa, [(64, nsp), (1, 64)]),
                    in0=_ap(D, [(1, nsp * 64)]),
                    in1=_ap(Sacc, [(1, nsp * 64)]), op=MULT)
                nc.vector.tensor_reduce(
                    out=_ap(LJs, [(1, nsp)]),
                    in_=_ap(Pa, [(64, nsp), (1, 64)]),
                    axis=mybir.AxisListType.X, op=ADD)
                nc.vector.tensor_tensor(
                    out=_ap(LJ, [(1, nsp)]),
                    in0=_ap(LJ, [(1, nsp)]),
                    in1=_ap(LJs, [(1, nsp)]), op=ADD)

                nc.sync.dma_start(
                    out=bass.AP(uo_d, off_u, [[64, 128], [8192, nsp], [1, 64]]),
                    in_=_ap(U, [(64, nsp), (1, 64)]))
                nc.sync.dma_start(
                    out=bass.AP(lj_d, off_l, [[1, 128], [128, nsp]]),
                    in_=_ap(LJ, [(1, nsp)]))

            if unroll:
                for ti in range(tiles):
                    body(ti)
            else:
                with tc.For_i(0, tiles) as ti:
                    body(ti)

    nc.compile()
    return nc


_CACHE = {}


def _host_consts(W0, W1, W2):
    W0a = 0.5 * (W0 - W0.T)
    W1a = 0.5 * (W1 - W1.T)
    Wa = 0.5 * (W2 - W2.T)
    gs = [W0a + (j * DT / 2.0) * W1a for j in range(33)]
    was = [((1.0, 2.0, 2.0, 1.0)[st] * DT / 6.0) * Wa for st in range(4)]
    return np.concatenate([Wa.ravel(), (-Wa).ravel()] +
                          [g.ravel() for g in gs] +
                          [w.ravel() for w in was]).astype(np.float32)


def kernel(U0, eps, W0, W1, W2):
    U0 = np.asarray(U0, dtype=np.float32)
    eps = np.asarray(eps, dtype=np.float32)
    W0 = np.asarray(W0, dtype=np.float32)
    W1 = np.asarray(W1, dtype=np.float32)
    W2 = np.asarray(W2, dtype=np.float32)

    if 'nc' not in _CACHE:
        _CACHE['nc'] = build()
    nc = _CACHE['nc']

    cst = _host_consts(W0, W1, W2)
    u_flat = U0.reshape(B, 64)
    e_flat = eps.reshape(B, 64)

    in_maps = []
    for c in range(N_CORES):
        sl = slice(c * SHARD, (c + 1) * SHARD)
        in_maps.append({"u0": np.ascontiguousarray(u_flat[sl]),
                        "ep": np.ascontiguousarray(e_flat[sl]),
                        "cst": cst})
    res = run_bass_kernel_spmd(nc, in_maps, core_ids=list(range(N_CORES)))
    U_out = np.concatenate([res.results[c]["uo"] for c in range(N_CORES)],
                           axis=0).reshape(B, 8, 8)
    lj_out = np.concatenate([res.results[c]["lj"] for c in range(N_CORES)],
                            axis=0)
    return U_out, lj_out


# revision 19
# speedup vs baseline: 1.0006x; 1.0006x over previous
"""Trainium2 Bass kernel for nn_AdjLieODEflow (batched 8x8 Lie-ODE RK4 flow).

Math (per sample, 8x8 matrices):
  dU/dt = F(t,U) U,  F = antisym(W0 + t W1 + U W2 U^T) = G(t) + U Wa U^T
  where G(t) = W0a + t W1a (antisym parts), Wa = antisym(W2).
  vel = G U + U (Wa S),  S = U^T U.
  logj rate (Hutchinson, v = eps[0], S=1):
    rate = tr(T Wa S) + tr(Q Wa Q),  T = v^T v,  Q = v^T U
    (the other two JVP-trace terms vanish exactly: <sym, antisym> = 0)
    tr(T Wa S) = <D, S>, D = T Wa (time-constant!) -> accumulated as
      <D, sum_t c_t S_t> with one dot at tile end (Sacc trick).
    tr(Q Wa Q) = <Q2, Q2^T (c Wa)>, Q2 = U^T v, RK weight c folded into Wa.
  RK4, 16 steps, dt = 1/16. Sharding: pure data parallel over 8 cores.

Layout: sample s = tile*2048 + q*128 + p (p = partition, q chunk 0..15).
Each sample's 8x8 matrices live in one partition's free dim (64 elems,
row-major). A per-sample matmul = 8 outer-product elementwise ops (one per
contraction index) into P[q, slot, 8, 8] + a 3-level binary tree of adds.

Everything runs on VectorE. Measured facts that force this design:
 - TensorE can't help: its contraction dim is the partition axis; placing
   per-sample data block-diagonally needs partition-dependent free offsets,
   which no AP can express (BIR: "illegal partition step"), and weight-load
   bandwidth would bottleneck anyway.
 - GpSimd tensor ops ~halve VectorE throughput while active (measured
   2.4x slowdown on independent data), so concurrent GPS offload reduces
   total throughput; GPS is left idle.
 - f32 tensor_tensor runs at 1 elem/cycle (no 2x mode); ops hit the
   formula cost (N+151)/0.96GHz only when the DVE runs alone.
 - bf16 gives 2x only on unit-stride tensor_tensor ops; using it for the
   S/H/UH chains blows the 2e-2 error budget (2.8e-2 measured full-size),
   so bf16 (Pb/Tb/Q2/WQn/velA/TG) covers only the logj chain and the
   additive velA = G*Ue term (U err 3.0e-4, logj err 4.6e-3, ~59 ms;
   chip power-state flips some runs to 1.2x faster ~49 ms).
"""
import sys
import types

sys.path.insert(0, "/opt/trn_rl_repo")
sys.path.insert(0, "/root/.axon_site")

import numpy as np


def _install_profile_hook():
    try:
        import antenv
        if 'antenv.axon_hooks' in sys.modules:
            return
        mod = types.ModuleType('antenv.axon_hooks')
        _h = [None]
        mod.set_axon_ntff_profile_hook = lambda h: _h.__setitem__(0, h)
        mod.get_axon_ntff_profile_hook = lambda: _h[0]
        sys.modules['antenv.axon_hooks'] = mod
        antenv.axon_hooks = mod
        from trn_agent_boot.trn_boot import _ntff_profile_via_ctypes
        hook = _ntff_profile_via_ctypes('/opt/axon/libaxon_pjrt.so')
        if hook is not None:
            mod.set_axon_ntff_profile_hook(hook)
    except Exception:
        pass


_install_profile_hook()

import concourse.bass as bass
import concourse.bacc as bacc
import concourse.tile as tile
from concourse import mybir
from concourse.bass_utils import run_bass_kernel_spmd

f32 = mybir.dt.float32
bf16 = mybir.dt.bfloat16
ADD = mybir.AluOpType.add
MULT = mybir.AluOpType.mult

B = 131072
N_CORES = 8
SHARD = B // N_CORES
NSP = 16
TS = 128 * NSP
N_STEPS = 16
DT = 1.0 / N_STEPS


def _ap(t, dims, off=0):
    base = t[:, :]
    return bass.AP(base.tensor, base.offset + off,
                   [[base.ap[0][0], 128]] + [list(d) for d in dims])


def build(n_cores=N_CORES, shard=SHARD, nsp=NSP, unroll=False,
          state_gps_slots=()):
    ts = 128 * nsp
    tiles = shard // ts
    assert tiles * ts == shard

    nc = bacc.Bacc("TRN2", target_bir_lowering=False, debug=False,
                   num_devices=n_cores)
    u0_d = nc.declare_dram_parameter("u0", [shard, 64], f32, isOutput=False)
    ep_d = nc.declare_dram_parameter("ep", [shard, 64], f32, isOutput=False)
    cst_d = nc.declare_dram_parameter("cst", [39 * 64], f32, isOutput=False)
    uo_d = nc.declare_dram_parameter("uo", [shard, 64], f32, isOutput=True)
    lj_d = nc.declare_dram_parameter("lj", [shard], f32, isOutput=True)

    with tile.TileContext(nc) as tc:
        with tc.tile_pool(name="w", bufs=1) as pool:
            CST = pool.tile([128, 39 * 64], f32)
            U = pool.tile([128, nsp * 64], f32)
            Vv = pool.tile([128, nsp * 64], f32)
            Ue0 = pool.tile([128, nsp * 64], f32)
            Ue1 = pool.tile([128, nsp * 64], f32)
            D = pool.tile([128, nsp * 64], f32)
            Sacc = pool.tile([128, nsp * 64], f32)
            S0 = pool.tile([128, nsp * 64], f32)
            S1 = pool.tile([128, nsp * 64], f32)
            Q2 = pool.tile([128, nsp * 64], bf16)
            H = pool.tile([128, nsp * 64], f32)
            WQn = pool.tile([128, nsp * 64], bf16)
            velA = pool.tile([128, nsp * 64], bf16)
            V1 = pool.tile([128, nsp * 64], f32)
            V2 = pool.tile([128, nsp * 64], f32)
            V3 = pool.tile([128, nsp * 64], f32)
            V4 = pool.tile([128, nsp * 64], f32)
            Aa = pool.tile([128, nsp * 64], f32)
            Ab = pool.tile([128, nsp * 64], f32)
            Pa = pool.tile([128, nsp * 512], f32)
            Ta = pool.tile([128, nsp * 256], f32)
            Pb = pool.tile([128, nsp * 512], bf16)
            Tb = pool.tile([128, nsp * 256], bf16)
            TG = pool.tile([128, nsp * 256], bf16)
            LJ = pool.tile([128, nsp], f32)
            LJs = pool.tile([128, nsp], f32)

            nc.sync.dma_start(out=CST[:, :],
                              in_=bass.AP(cst_d, 0, [[0, 128], [1, 39 * 64]]))

            WA_OFF = 0
            G_OFF = 128
            WAS_OFF = 35 * 64

            def prod_aps(kind, Pbuf, k, inA, inB, offA):
                # P layout [half(2), q, slot'(4), 8, 8]: add1 is fully flat
                out = _ap(Pbuf, [(256, nsp), (8, 8), (1, 8)],
                          off=(k % 4) * 64 + (k // 4) * nsp * 256)
                if kind == 'rowrow':
                    a = _ap(inA, [(64, nsp), (1, 8), (0, 8)], off=8 * k)
                    b = _ap(inB, [(64, nsp), (0, 8), (1, 8)], off=8 * k)
                elif kind == 'constL':
                    a = _ap(inA, [(0, nsp), (8, 8), (0, 8)], off=offA + k)
                    b = _ap(inB, [(64, nsp), (0, 8), (1, 8)], off=8 * k)
                elif kind == 'dataL':
                    a = _ap(inA, [(64, nsp), (8, 8), (0, 8)], off=k)
                    b = _ap(inB, [(64, nsp), (0, 8), (1, 8)], off=8 * k)
                elif kind == 'colconstR':
                    a = _ap(inA, [(64, nsp), (1, 8), (0, 8)], off=8 * k)
                    b = _ap(inB, [(0, nsp), (0, 8), (1, 8)], off=offA + 8 * k)
                elif kind == 'dataLconstR':
                    a = _ap(inA, [(64, nsp), (8, 8), (0, 8)], off=k)
                    b = _ap(inB, [(0, nsp), (0, 8), (1, 8)], off=offA + 8 * k)
                return out, a, b

            def tree(e, Pbuf, Tbuf, out, plus=None):
                # add1: halves are nsp*256 apart and fully contiguous
                e.tensor_tensor(
                    out=_ap(Tbuf, [(1, nsp * 256)]),
                    in0=_ap(Pbuf, [(1, nsp * 256)], off=0),
                    in1=_ap(Pbuf, [(1, nsp * 256)], off=nsp * 256), op=ADD)
                e.tensor_tensor(
                    out=_ap(Pbuf, [(256, nsp), (1, 128)], off=0),
                    in0=_ap(Tbuf, [(256, nsp), (1, 128)], off=0),
                    in1=_ap(Tbuf, [(256, nsp), (1, 128)], off=128), op=ADD)
                if plus is None:
                    e.tensor_tensor(
                        out=_ap(out, [(64, nsp), (1, 64)]),
                        in0=_ap(Pbuf, [(256, nsp), (1, 64)], off=0),
                        in1=_ap(Pbuf, [(256, nsp), (1, 64)], off=64), op=ADD)
                else:
                    e.tensor_tensor(
                        out=_ap(Tbuf, [(256, nsp), (1, 64)]),
                        in0=_ap(Pbuf, [(256, nsp), (1, 64)], off=0),
                        in1=_ap(Pbuf, [(256, nsp), (1, 64)], off=64), op=ADD)
                    e.tensor_tensor(
                        out=_ap(out, [(64, nsp), (1, 64)]),
                        in0=_ap(Tbuf, [(256, nsp), (1, 64)]),
                        in1=_ap(plus, [(1, nsp * 64)]), op=ADD)

            def dve_group(kind, inA, inB, out, offA=0, plus=None,
                          gps_slots=(), buf=None):
                Pbuf, Tbuf = (Pa, Ta) if buf is None else buf
                for k in range(8):
                    o, a, b = prod_aps(kind, Pbuf, k, inA, inB, offA)
                    e = nc.gpsimd if k in gps_slots else nc.vector
                    e.tensor_tensor(out=o, in0=a, in1=b, op=MULT)
                tree(nc.vector, Pbuf, Tbuf, out, plus=plus)

            def gps_group(kind, inA, inB, out, offA=0):
                for k in range(8):
                    o, a, b = prod_aps(kind, Pb, k, inA, inB, offA)
                    nc.vector.tensor_tensor(out=o, in0=a, in1=b, op=MULT)
                tree(nc.vector, Pb, Tb, out)

            def emit_lj_step():
                nc.vector.tensor_reduce(
                    out=_ap(LJs, [(1, nsp)]),
                    in_=_ap(TG, [(256, nsp), (64, 4), (1, 64)]),
                    axis=mybir.AxisListType.XY, op=ADD)
                nc.vector.tensor_tensor(
                    out=_ap(LJ, [(1, nsp)]),
                    in0=_ap(LJ, [(1, nsp)]),
                    in1=_ap(LJs, [(1, nsp)]), op=ADD)

            def body(ti):
                off_u = ti * (ts * 64)
                off_l = ti * ts
                nc.sync.dma_start(
                    out=_ap(U, [(64, nsp), (1, 64)]),
                    in_=bass.AP(u0_d, off_u, [[64, 128], [8192, nsp], [1, 64]]))
                nc.sync.dma_start(
                    out=_ap(Vv, [(64, nsp), (1, 64)]),
                    in_=bass.AP(ep_d, off_u, [[64, 128], [8192, nsp], [1, 64]]))
                nc.vector.memset(_ap(LJ, [(1, nsp)]), 0.0)
                nc.vector.memset(_ap(Sacc, [(1, nsp * 64)]), 0.0)

                # setup: T = v^T v (H temp), D = T Wa   (GPS, it is idle)
                gps_group('rowrow', Vv, Vv, H)
                gps_group('dataLconstR', H, CST, D, offA=WA_OFF)

                pending = None   # (Ue_tile_or_U, S_tile, st) for logj chain

                def logj_chain(ue_t, s_t, st):
                    gps_group('rowrow', ue_t, Vv, Q2)
                    gps_group('colconstR', Q2, CST, WQn,
                              offA=WAS_OFF + 64 * st)
                    nc.vector.tensor_tensor(
                        out=_ap(TG, [(256, nsp), (1, 64)], off=64 * st),
                        in0=_ap(Q2, [(1, nsp * 64)]),
                        in1=_ap(WQn, [(1, nsp * 64)]), op=MULT)

                for step in range(N_STEPS):
                    for st in range(4):
                        par = (4 * step + st) % 2
                        Scur = (S0, S1)[par]
                        gi = 2 * step + (0 if st == 0 else (1 if st < 3 else 2))
                        if st == 0:
                            Ucur = U
                        else:
                            c = 0.5 * DT if st < 3 else DT
                            Vprev = (V1, V2, V3)[st - 1]
                            Ucur = (Ue0, Ue1)[par]
                            nc.vector.scalar_tensor_tensor(
                                out=_ap(Ucur, [(1, nsp * 64)]),
                                in0=_ap(Vprev, [(1, nsp * 64)]),
                                scalar=float(c),
                                in1=_ap(U, [(1, nsp * 64)]),
                                op0=MULT, op1=ADD)
                        Vst = (V1, V2, V3, V4)[st]

                        dve_group('rowrow', Ucur, Ucur, Scur,
                                  gps_slots=state_gps_slots)
                        dve_group('constL', CST, Scur, H, offA=WA_OFF)
                        dve_group('constL', CST, Ucur, velA,
                                  offA=G_OFF + 64 * gi,
                                  gps_slots=state_gps_slots, buf=(Pb, Tb))
                        dve_group('dataL', Ucur, H, Vst, plus=velA)

                        # term1 accumulation (D const):  Sacc += c_st * S
                        cw = (1.0, 2.0, 2.0, 1.0)[st] * DT / 6.0
                        nc.vector.scalar_tensor_tensor(
                            out=_ap(Sacc, [(1, nsp * 64)]),
                            in0=_ap(Scur, [(1, nsp * 64)]),
                            scalar=float(cw),
                            in1=_ap(Sacc, [(1, nsp * 64)]),
                            op0=MULT, op1=ADD)

                        if st == 1:
                            nc.vector.scalar_tensor_tensor(
                                out=_ap(Aa, [(1, nsp * 64)]),
                                in0=_ap(V2, [(1, nsp * 64)]), scalar=2.0,
                                in1=_ap(V1, [(1, nsp * 64)]),
                                op0=MULT, op1=ADD)
                        elif st == 2:
                            nc.vector.scalar_tensor_tensor(
                                out=_ap(Ab, [(1, nsp * 64)]),
                                in0=_ap(V3, [(1, nsp * 64)]), scalar=2.0,
                                in1=_ap(Aa, [(1, nsp * 64)]),
                                op0=MULT, op1=ADD)
                        elif st == 3:
                            nc.vector.tensor_tensor(
                                out=_ap(Aa, [(1, nsp * 64)]),
                                in0=_ap(V4, [(1, nsp * 64)]),
                                in1=_ap(Ab, [(1, nsp * 64)]), op=ADD)
                            nc.vector.scalar_tensor_tensor(
                                out=_ap(U, [(1, nsp * 64)]),
                                in0=_ap(Aa, [(1, nsp * 64)]),
                                scalar=float(DT / 6.0),
                                in1=_ap(U, [(1, nsp * 64)]),
                                op0=MULT, op1=ADD)

                        # previous step's logj reduce, before slot-0 rewrite
                        if st == 1 and step > 0:
                            emit_lj_step()
                        # lagged logj chain for the previous stage
                        if pending is not None:
                            logj_chain(*pending)
                        pending = (Ucur, Scur, st)

                logj_chain(*pending)
                emit_lj_step()
                # term1 total: LJ += sum(D o Sacc)
                nc.vector.tensor_tensor(
                    out=_ap(Pa, [(64, nsp), (1, 64)]),
                    in0=_ap(D, [(1, nsp * 64)]),
                    in1=_ap(Sacc, [(1, nsp * 64)]), op=MULT)
                nc.vector.tensor_reduce(
                    out=_ap(LJs, [(1, nsp)]),
                    in_=_ap(Pa, [(64, nsp), (1, 64)]),
                    axis=mybir.AxisListType.X, op=ADD)
                nc.vector.tensor_tensor(
                    out=_ap(LJ, [(1, nsp)]),
                    in0=_ap(LJ, [(1, nsp)]),
                    in1=_ap(LJs, [(1, nsp)]), op=ADD)

                nc.sync.dma_start(
                    out=bass.AP(uo_d, off_u, [[64, 128], [8192, nsp], [1, 64]]),
                    in_=_ap(U, [(64, nsp), (1, 64)]))
                nc.sync.dma_start(
                    out=bass.AP(lj_d, off_l, [[1, 128], [128, nsp]]),
                    in_=_ap(LJ, [(1, nsp)]))

            if unroll:
                for ti in range(tiles):
                    body(ti)
            else:
                with tc.For_i(0, tiles) as ti:
                    body(ti)

    nc.compile()
    return nc


_CACHE = {}


def _host_consts(W0, W1, W2):
    W0a = 0.5 * (W0 - W0.T)
    W1a = 0.5 * (W1 - W1.T)
    Wa = 0.5 * (W2 - W2.T)
    gs = [W0a + (j * DT / 2.0) * W1a for j in range(33)]
    was = [((1.0, 2.0, 2.0, 1.0)[st] * DT / 6.0) * Wa for st in range(4)]
    return np.concatenate([Wa.ravel(), (-Wa).ravel()] +
                          [g.ravel() for g in gs] +
                          [w.ravel() for w in was]).astype(np.float32)


def kernel(U0, eps, W0, W1, W2):
    U0 = np.asarray(U0, dtype=np.float32)
    eps = np.asarray(eps, dtype=np.float32)
    W0 = np.asarray(W0, dtype=np.float32)
    W1 = np.asarray(W1, dtype=np.float32)
    W2 = np.asarray(W2, dtype=np.float32)

    if 'nc' not in _CACHE:
        _CACHE['nc'] = build()
    nc = _CACHE['nc']

    cst = _host_consts(W0, W1, W2)
    u_flat = U0.reshape(B, 64)
    e_flat = eps.reshape(B, 64)

    in_maps = []
    for c in range(N_CORES):
        sl = slice(c * SHARD, (c + 1) * SHARD)
        in_maps.append({"u0": np.ascontiguousarray(u_flat[sl]),
                        "ep": np.ascontiguousarray(e_flat[sl]),
                        "cst": cst})
    res = run_bass_kernel_spmd(nc, in_maps, core_ids=list(range(N_CORES)))
    U_out = np.concatenate([res.results[c]["uo"] for c in range(N_CORES)],
                           axis=0).reshape(B, 8, 8)
    lj_out = np.concatenate([res.results[c]["lj"] for c in range(N_CORES)],
                            axis=0)
    return U_out, lj_out


# revision 20
# speedup vs baseline: 1.1183x; 1.1176x over previous
"""Trainium2 Bass kernel for nn_AdjLieODEflow (batched 8x8 Lie-ODE RK4 flow).

Math (per sample, 8x8 matrices):
  dU/dt = F(t,U) U,  F = antisym(W0 + t W1 + U W2 U^T) = G(t) + U Wa U^T
  where G(t) = W0a + t W1a (antisym parts), Wa = antisym(W2).
  vel = G U + U (Wa S),  S = U^T U.
  logj rate (Hutchinson, v = eps[0], S=1):
    rate = tr(T Wa S) + tr(Q Wa Q),  T = v^T v,  Q = v^T U
    (the other two JVP-trace terms vanish exactly: <sym, antisym> = 0)
    tr(T Wa S) = <D, S>, D = T Wa (time-constant!) -> accumulated as
      <D, sum_t c_t S_t> with one dot at tile end (Sacc trick).
    tr(Q Wa Q) = <Q2, Q2^T (c Wa)>, Q2 = U^T v, RK weight c folded into Wa.
  RK4, 16 steps, dt = 1/16. Sharding: pure data parallel over 8 cores.

Layout: sample s = tile*2048 + q*128 + p (p = partition, q chunk 0..15).
Each sample's 8x8 matrices live in one partition's free dim (64 elems,
row-major). A per-sample matmul = 8 outer-product elementwise ops (one per
contraction index) into P[q, slot, 8, 8] + a 3-level binary tree of adds.

Everything runs on VectorE. Measured facts that force this design:
 - TensorE can't help: its contraction dim is the partition axis; placing
   per-sample data block-diagonally needs partition-dependent free offsets,
   which no AP can express (BIR: "illegal partition step"), and weight-load
   bandwidth would bottleneck anyway.
 - GpSimd tensor ops ~halve VectorE throughput while active (measured
   2.4x slowdown on independent data), so concurrent GPS offload reduces
   total throughput; GPS is left idle.
 - f32 tensor_tensor runs at 1 elem/cycle (no 2x mode); ops hit the
   formula cost (N+151)/0.96GHz only when the DVE runs alone.
 - bf16 gives 2x only on unit-stride tensor_tensor ops; using it for the
   S/H/UH chains blows the 2e-2 error budget (2.8e-2 measured full-size),
   so bf16 (Pb/Tb/Q2/WQn/velA/TG) covers only the logj chain and the
   additive velA = G*Ue term (U err 3.0e-4, logj err 4.6e-3, ~59 ms;
   chip power-state flips some runs to 1.2x faster ~49 ms).
"""
import sys
import types

sys.path.insert(0, "/opt/trn_rl_repo")
sys.path.insert(0, "/root/.axon_site")

import numpy as np


def _install_profile_hook():
    try:
        import antenv
        if 'antenv.axon_hooks' in sys.modules:
            return
        mod = types.ModuleType('antenv.axon_hooks')
        _h = [None]
        mod.set_axon_ntff_profile_hook = lambda h: _h.__setitem__(0, h)
        mod.get_axon_ntff_profile_hook = lambda: _h[0]
        sys.modules['antenv.axon_hooks'] = mod
        antenv.axon_hooks = mod
        from trn_agent_boot.trn_boot import _ntff_profile_via_ctypes
        hook = _ntff_profile_via_ctypes('/opt/axon/libaxon_pjrt.so')
        if hook is not None:
            mod.set_axon_ntff_profile_hook(hook)
    except Exception:
        pass


_install_profile_hook()

import concourse.bass as bass
import concourse.bacc as bacc
import concourse.tile as tile
from concourse import mybir
from concourse.bass_utils import run_bass_kernel_spmd

f32 = mybir.dt.float32
bf16 = mybir.dt.bfloat16
fp16 = mybir.dt.float16
ADD = mybir.AluOpType.add
MULT = mybir.AluOpType.mult

B = 131072
N_CORES = 8
SHARD = B // N_CORES
NSP = 16
TS = 128 * NSP
N_STEPS = 16
DT = 1.0 / N_STEPS


def _ap(t, dims, off=0):
    base = t[:, :]
    return bass.AP(base.tensor, base.offset + off,
                   [[base.ap[0][0], 128]] + [list(d) for d in dims])


def build(n_cores=N_CORES, shard=SHARD, nsp=NSP, unroll=False,
          state_gps_slots=()):
    ts = 128 * nsp
    tiles = shard // ts
    assert tiles * ts == shard

    nc = bacc.Bacc("TRN2", target_bir_lowering=False, debug=False,
                   num_devices=n_cores)
    u0_d = nc.declare_dram_parameter("u0", [shard, 64], f32, isOutput=False)
    ep_d = nc.declare_dram_parameter("ep", [shard, 64], f32, isOutput=False)
    cst_d = nc.declare_dram_parameter("cst", [39 * 64], f32, isOutput=False)
    uo_d = nc.declare_dram_parameter("uo", [shard, 64], f32, isOutput=True)
    lj_d = nc.declare_dram_parameter("lj", [shard], f32, isOutput=True)

    with tile.TileContext(nc) as tc:
        with tc.tile_pool(name="w", bufs=1) as pool:
            CST = pool.tile([128, 39 * 64], f32)
            U = pool.tile([128, nsp * 64], f32)
            Vv = pool.tile([128, nsp * 64], f32)
            Ue0 = pool.tile([128, nsp * 64], f32)
            Ue1 = pool.tile([128, nsp * 64], f32)
            D = pool.tile([128, nsp * 64], f32)
            Sacc = pool.tile([128, nsp * 64], f32)
            S0 = pool.tile([128, nsp * 64], f32)
            S1 = pool.tile([128, nsp * 64], f32)
            Q2 = pool.tile([128, nsp * 64], bf16)
            H = pool.tile([128, nsp * 64], f32)
            WQn = pool.tile([128, nsp * 64], bf16)
            velA = pool.tile([128, nsp * 64], bf16)
            V1 = pool.tile([128, nsp * 64], f32)
            V2 = pool.tile([128, nsp * 64], f32)
            V3 = pool.tile([128, nsp * 64], f32)
            V4 = pool.tile([128, nsp * 64], f32)
            Aa = pool.tile([128, nsp * 64], f32)
            Ab = pool.tile([128, nsp * 64], f32)
            Pa = pool.tile([128, nsp * 512], fp16)
            Ta = pool.tile([128, nsp * 256], fp16)
            Pb = pool.tile([128, nsp * 512], bf16)
            Tb = pool.tile([128, nsp * 256], bf16)
            TG = pool.tile([128, nsp * 256], bf16)
            LJ = pool.tile([128, nsp], f32)
            LJs = pool.tile([128, nsp], f32)

            nc.sync.dma_start(out=CST[:, :],
                              in_=bass.AP(cst_d, 0, [[0, 128], [1, 39 * 64]]))

            WA_OFF = 0
            G_OFF = 128
            WAS_OFF = 35 * 64

            def prod_aps(kind, Pbuf, k, inA, inB, offA):
                # P layout [half(2), q, slot'(4), 8, 8]: add1 is fully flat
                out = _ap(Pbuf, [(256, nsp), (8, 8), (1, 8)],
                          off=(k % 4) * 64 + (k // 4) * nsp * 256)
                if kind == 'rowrow':
                    a = _ap(inA, [(64, nsp), (1, 8), (0, 8)], off=8 * k)
                    b = _ap(inB, [(64, nsp), (0, 8), (1, 8)], off=8 * k)
                elif kind == 'constL':
                    a = _ap(inA, [(0, nsp), (8, 8), (0, 8)], off=offA + k)
                    b = _ap(inB, [(64, nsp), (0, 8), (1, 8)], off=8 * k)
                elif kind == 'dataL':
                    a = _ap(inA, [(64, nsp), (8, 8), (0, 8)], off=k)
                    b = _ap(inB, [(64, nsp), (0, 8), (1, 8)], off=8 * k)
                elif kind == 'colconstR':
                    a = _ap(inA, [(64, nsp), (1, 8), (0, 8)], off=8 * k)
                    b = _ap(inB, [(0, nsp), (0, 8), (1, 8)], off=offA + 8 * k)
                elif kind == 'dataLconstR':
                    a = _ap(inA, [(64, nsp), (8, 8), (0, 8)], off=k)
                    b = _ap(inB, [(0, nsp), (0, 8), (1, 8)], off=offA + 8 * k)
                return out, a, b

            def tree(e, Pbuf, Tbuf, out, plus=None):
                # add1: halves are nsp*256 apart and fully contiguous
                e.tensor_tensor(
                    out=_ap(Tbuf, [(1, nsp * 256)]),
                    in0=_ap(Pbuf, [(1, nsp * 256)], off=0),
                    in1=_ap(Pbuf, [(1, nsp * 256)], off=nsp * 256), op=ADD)
                e.tensor_tensor(
                    out=_ap(Pbuf, [(256, nsp), (1, 128)], off=0),
                    in0=_ap(Tbuf, [(256, nsp), (1, 128)], off=0),
                    in1=_ap(Tbuf, [(256, nsp), (1, 128)], off=128), op=ADD)
                if plus is None:
                    e.tensor_tensor(
                        out=_ap(out, [(64, nsp), (1, 64)]),
                        in0=_ap(Pbuf, [(256, nsp), (1, 64)], off=0),
                        in1=_ap(Pbuf, [(256, nsp), (1, 64)], off=64), op=ADD)
                else:
                    e.tensor_tensor(
                        out=_ap(Tbuf, [(256, nsp), (1, 64)]),
                        in0=_ap(Pbuf, [(256, nsp), (1, 64)], off=0),
                        in1=_ap(Pbuf, [(256, nsp), (1, 64)], off=64), op=ADD)
                    e.tensor_tensor(
                        out=_ap(out, [(64, nsp), (1, 64)]),
                        in0=_ap(Tbuf, [(256, nsp), (1, 64)]),
                        in1=_ap(plus, [(1, nsp * 64)]), op=ADD)

            def dve_group(kind, inA, inB, out, offA=0, plus=None,
                          gps_slots=(), buf=None):
                Pbuf, Tbuf = (Pa, Ta) if buf is None else buf
                for k in range(8):
                    o, a, b = prod_aps(kind, Pbuf, k, inA, inB, offA)
                    e = nc.gpsimd if k in gps_slots else nc.vector
                    e.tensor_tensor(out=o, in0=a, in1=b, op=MULT)
                tree(nc.vector, Pbuf, Tbuf, out, plus=plus)

            def gps_group(kind, inA, inB, out, offA=0):
                for k in range(8):
                    o, a, b = prod_aps(kind, Pb, k, inA, inB, offA)
                    nc.vector.tensor_tensor(out=o, in0=a, in1=b, op=MULT)
                tree(nc.vector, Pb, Tb, out)

            def emit_lj_step():
                nc.vector.tensor_reduce(
                    out=_ap(LJs, [(1, nsp)]),
                    in_=_ap(TG, [(256, nsp), (64, 4), (1, 64)]),
                    axis=mybir.AxisListType.XY, op=ADD)
                nc.vector.tensor_tensor(
                    out=_ap(LJ, [(1, nsp)]),
                    in0=_ap(LJ, [(1, nsp)]),
                    in1=_ap(LJs, [(1, nsp)]), op=ADD)

            def body(ti):
                off_u = ti * (ts * 64)
                off_l = ti * ts
                nc.sync.dma_start(
                    out=_ap(U, [(64, nsp), (1, 64)]),
                    in_=bass.AP(u0_d, off_u, [[64, 128], [8192, nsp], [1, 64]]))
                nc.sync.dma_start(
                    out=_ap(Vv, [(64, nsp), (1, 64)]),
                    in_=bass.AP(ep_d, off_u, [[64, 128], [8192, nsp], [1, 64]]))
                nc.vector.memset(_ap(LJ, [(1, nsp)]), 0.0)
                nc.vector.memset(_ap(Sacc, [(1, nsp * 64)]), 0.0)

                # setup: T = v^T v (H temp), D = T Wa   (GPS, it is idle)
                gps_group('rowrow', Vv, Vv, H)
                gps_group('dataLconstR', H, CST, D, offA=WA_OFF)

                pending = None   # (Ue_tile_or_U, S_tile, st) for logj chain

                def logj_chain(ue_t, s_t, st):
                    gps_group('rowrow', ue_t, Vv, Q2)
                    gps_group('colconstR', Q2, CST, WQn,
                              offA=WAS_OFF + 64 * st)
                    nc.vector.tensor_tensor(
                        out=_ap(TG, [(256, nsp), (1, 64)], off=64 * st),
                        in0=_ap(Q2, [(1, nsp * 64)]),
                        in1=_ap(WQn, [(1, nsp * 64)]), op=MULT)

                for step in range(N_STEPS):
                    for st in range(4):
                        par = (4 * step + st) % 2
                        Scur = (S0, S1)[par]
                        gi = 2 * step + (0 if st == 0 else (1 if st < 3 else 2))
                        if st == 0:
                            Ucur = U
                        else:
                            c = 0.5 * DT if st < 3 else DT
                            Vprev = (V1, V2, V3)[st - 1]
                            Ucur = (Ue0, Ue1)[par]
                            nc.vector.scalar_tensor_tensor(
                                out=_ap(Ucur, [(1, nsp * 64)]),
                                in0=_ap(Vprev, [(1, nsp * 64)]),
                                scalar=float(c),
                                in1=_ap(U, [(1, nsp * 64)]),
                                op0=MULT, op1=ADD)
                        Vst = (V1, V2, V3, V4)[st]

                        dve_group('rowrow', Ucur, Ucur, Scur,
                                  gps_slots=state_gps_slots)
                        dve_group('constL', CST, Scur, H, offA=WA_OFF)
                        dve_group('constL', CST, Ucur, velA,
                                  offA=G_OFF + 64 * gi,
                                  gps_slots=state_gps_slots, buf=(Pb, Tb))
                        dve_group('dataL', Ucur, H, Vst, plus=velA)

                        # term1 accumulation (D const):  Sacc += c_st * S
                        cw = (1.0, 2.0, 2.0, 1.0)[st] * DT / 6.0
                        nc.vector.scalar_tensor_tensor(
                            out=_ap(Sacc, [(1, nsp * 64)]),
                            in0=_ap(Scur, [(1, nsp * 64)]),
                            scalar=float(cw),
                            in1=_ap(Sacc, [(1, nsp * 64)]),
                            op0=MULT, op1=ADD)

                        if st == 1:
                            nc.vector.scalar_tensor_tensor(
                                out=_ap(Aa, [(1, nsp * 64)]),
                                in0=_ap(V2, [(1, nsp * 64)]), scalar=2.0,
                                in1=_ap(V1, [(1, nsp * 64)]),
                                op0=MULT, op1=ADD)
                        elif st == 2:
                            nc.vector.scalar_tensor_tensor(
                                out=_ap(Ab, [(1, nsp * 64)]),
                                in0=_ap(V3, [(1, nsp * 64)]), scalar=2.0,
                                in1=_ap(Aa, [(1, nsp * 64)]),
                                op0=MULT, op1=ADD)
                        elif st == 3:
                            nc.vector.tensor_tensor(
                                out=_ap(Aa, [(1, nsp * 64)]),
                                in0=_ap(V4, [(1, nsp * 64)]),
                                in1=_ap(Ab, [(1, nsp * 64)]), op=ADD)
                            nc.vector.scalar_tensor_tensor(
                                out=_ap(U, [(1, nsp * 64)]),
                                in0=_ap(Aa, [(1, nsp * 64)]),
                                scalar=float(DT / 6.0),
                                in1=_ap(U, [(1, nsp * 64)]),
                                op0=MULT, op1=ADD)

                        # previous step's logj reduce, before slot-0 rewrite
                        if st == 1 and step > 0:
                            emit_lj_step()
                        # lagged logj chain for the previous stage
                        if pending is not None:
                            logj_chain(*pending)
                        pending = (Ucur, Scur, st)

                logj_chain(*pending)
                emit_lj_step()
                # term1 total: LJ += sum(D o Sacc)
                nc.vector.tensor_tensor(
                    out=_ap(Pa, [(64, nsp), (1, 64)]),
                    in0=_ap(D, [(1, nsp * 64)]),
                    in1=_ap(Sacc, [(1, nsp * 64)]), op=MULT)
                nc.vector.tensor_reduce(
                    out=_ap(LJs, [(1, nsp)]),
                    in_=_ap(Pa, [(64, nsp), (1, 64)]),
                    axis=mybir.AxisListType.X, op=ADD)
                nc.vector.tensor_tensor(
                    out=_ap(LJ, [(1, nsp)]),
                    in0=_ap(LJ, [(1, nsp)]),
                    in1=_ap(LJs, [(1, nsp)]), op=ADD)

                nc.sync.dma_start(
                    out=bass.AP(uo_d, off_u, [[64, 128], [8192, nsp], [1, 64]]),
                    in_=_ap(U, [(64, nsp), (1, 64)]))
                nc.sync.dma_start(
                    out=bass.AP(lj_d, off_l, [[1, 128], [128, nsp]]),
                    in_=_ap(LJ, [(1, nsp)]))

            if unroll:
                for ti in range(tiles):
                    body(ti)
            else:
                with tc.For_i(0, tiles) as ti:
                    body(ti)

    nc.compile()
    return nc


_CACHE = {}


def _host_consts(W0, W1, W2):
    W0a = 0.5 * (W0 - W0.T)
    W1a = 0.5 * (W1 - W1.T)
    Wa = 0.5 * (W2 - W2.T)
    gs = [W0a + (j * DT / 2.0) * W1a for j in range(33)]
    was = [((1.0, 2.0, 2.0, 1.0)[st] * DT / 6.0) * Wa for st in range(4)]
    return np.concatenate([Wa.ravel(), (-Wa).ravel()] +
                          [g.ravel() for g in gs] +
                          [w.ravel() for w in was]).astype(np.float32)


def kernel(U0, eps, W0, W1, W2):
    U0 = np.asarray(U0, dtype=np.float32)
    eps = np.asarray(eps, dtype=np.float32)
    W0 = np.asarray(W0, dtype=np.float32)
    W1 = np.asarray(W1, dtype=np.float32)
    W2 = np.asarray(W2, dtype=np.float32)

    if 'nc' not in _CACHE:
        _CACHE['nc'] = build()
    nc = _CACHE['nc']

    cst = _host_consts(W0, W1, W2)
    u_flat = U0.reshape(B, 64)
    e_flat = eps.reshape(B, 64)

    in_maps = []
    for c in range(N_CORES):
        sl = slice(c * SHARD, (c + 1) * SHARD)
        in_maps.append({"u0": np.ascontiguousarray(u_flat[sl]),
                        "ep": np.ascontiguousarray(e_flat[sl]),
                        "cst": cst})
    res = run_bass_kernel_spmd(nc, in_maps, core_ids=list(range(N_CORES)))
    U_out = np.concatenate([res.results[c]["uo"] for c in range(N_CORES)],
                           axis=0).reshape(B, 8, 8)
    lj_out = np.concatenate([res.results[c]["lj"] for c in range(N_CORES)],
                            axis=0)
    return U_out, lj_out
